# revision 1
# baseline (speedup 1.0000x reference)
"""Trainium2 Bass kernel for few-shot video retrieval (bidirectional chamfer
distance to class prototypes, global frame-level + segment-level, fused).

Contract: kernel(**inputs) takes the FULL unsharded inputs (numpy) and returns
the full outputs (tuple of 4 [4096, 64] float32 arrays), matching reference().

Sharding: data-parallel over the query axis across 8 NeuronCores; support
features / labels / fusion params replicated. Gather on host by concatenation.

Device-side algorithm per core (512 queries = 4 slices of 128):
  - all GEMM operands are fp8 e4m3; the main sims GEMM runs in DoubleRow mode
    (256-deep contraction per matmul), the segment GEMM in plain fp8 (FWL)
  - host pre-transposes queries to d-major layout, so no PE transposes at all
  - class prototypes via one-hot matmuls (contract the support dim on the PE,
    result is d-major = already in rhs layout); normalization multiplies use
    host-provided replicated 16/||proto|| rows
  - per-(q,t) 1/(16||q||) scales are folded into the ACT PSUM->SBUF copies
  - chamfer min/sum reductions = bf16 max/add halving trees on the DVE
    (tensor_tensor runs 2x on bf16; grouped tensor_reduce would be 1x)
"""

import sys

sys.path.insert(0, "/opt/trn_rl_repo")

import numpy as np
import ml_dtypes
from contextlib import ExitStack

import concourse.bass as bass
import concourse.bacc as bacc
import concourse.tile as tile
from concourse import mybir
from concourse.bass_utils import run_bass_kernel_spmd

# ---------------------------------------------------------------- problem dims
S, Q, T, D = 256, 4096, 8, 1024
K = 64                      # classes
NCORES = 8
QPC = Q // NCORES           # 512 queries per core
G = QPC // 128              # 4 query-slices of 128 per core
DCH = D // 128              # 8 chunks of the feature dim
DJ = DCH // 2               # 4 DoubleRow chunks (256-deep)
NW = 3                      # segment windows
WINDOWS = ((0, 4), (2, 6), (4, 8))
SCH = S // 128              # 2 support chunks
FSCALE = 16.0               # fp8 range scale folded into the norm factors

F32 = mybir.dt.float32
BF16 = mybir.dt.bfloat16
F8 = mybir.dt.float8e4
I32 = mybir.dt.int32
AF = mybir.ActivationFunctionType
ALU = mybir.AluOpType
AX = mybir.AxisListType
DR = mybir.MatmulPerfMode.DoubleRow

NP_F8 = ml_dtypes.float8_e4m3
NP_BF16 = ml_dtypes.bfloat16


# ---------------------------------------------------------------- bass kernel
def build_nc():
    nc = bacc.Bacc("TRN2", target_bir_lowering=False, debug=False,
                   num_devices=NCORES)

    # per-slice d-major queries: [p(d%128), j(d//256), o((d//128)%2), t, q]
    tf = nc.dram_tensor("tf", [G, 128, DJ * 2 * T * 128], F8,
                        kind="ExternalInput")
    # per-slice d-major window-summed segments: [p, dch, w, q]
    tseg = nc.dram_tensor("tseg", [G, 128, DCH * NW * 128], F8,
                          kind="ExternalInput")
    # s-major support: [c, s, (dch, t, d%128)]
    sf = nc.dram_tensor("sf", [SCH, 128, T * D], F8, kind="ExternalInput")
    # packed per-partition f32 constants (everything replicated per row):
    # [lab0, lab1, rqv(g*8+t: 32), rqs(g*3+w: 12), kiota(64), fus(3), lsc(1)]
    NSF = 2 + G * T + G * NW + K + 4
    smallf = nc.dram_tensor("smallf", [128, NSF], F32, kind="ExternalInput")
    # packed bf16 rows: [rprep(512) | rpsrep(192)]
    smallb = nc.dram_tensor("smallb", [128, T * K + NW * K], BF16,
                            kind="ExternalInput")

    # one contiguous output blob: [p, (which(4), g(4), k(64))]
    oall = nc.dram_tensor("oall", [128, 4 * G * K], F32,
                          kind="ExternalOutput")

    NWARM = 16

    with tile.TileContext(nc) as tc, ExitStack() as ctx:
        const = ctx.enter_context(tc.tile_pool(name="const", bufs=1))
        persist = ctx.enter_context(tc.tile_pool(name="persist", bufs=1))
        work = ctx.enter_context(tc.tile_pool(name="work", bufs=3))

        # ---------------- PE warmup burst: flips the HAM clock gate to 8/8
        # while the input DMAs are still in flight (results never read)
        wz = const.tile([128, 512], F8)
        nc.gpsimd.memset(wz[:], 0)
        psW = ctx.enter_context(tc.tile_pool(name="psW", bufs=1,
                                             space="PSUM"))
        wps = psW.tile([128, 512], F32)
        for _ in range(NWARM):
            nc.tensor.matmul(wps[:], wz[:, 0:128], wz[:], start=True,
                             stop=True)

        # ---------------- small constants, then support split across BOTH
        # DMA rings (support chunk 1 rides the sync ring in parallel)
        smallf_t = const.tile([128, NSF], F32)
        nc.gpsimd.dma_start(smallf_t[:], smallf[:])
        smallb_t = const.tile([128, T * K + NW * K], BF16)
        nc.gpsimd.dma_start(smallb_t[:], smallb[:])
        lab_t = [smallf_t[:, c:c + 1] for c in range(SCH)]
        rqv_t = [smallf_t[:, 2 + g * T:2 + (g + 1) * T] for g in range(G)]
        rqs_t = [smallf_t[:, 2 + G * T + g * NW:2 + G * T + (g + 1) * NW]
                 for g in range(G)]
        kbase = 2 + G * T + G * NW
        kiota_f = smallf_t[:, kbase:kbase + K]
        fus_t = smallf_t[:, kbase + K:kbase + K + 3]
        lsc_t = smallf_t[:, kbase + K + 3:kbase + K + 4]
        rprep_t = smallb_t[:, 0:T * K]
        rpsrep_t = smallb_t[:, T * K:T * K + NW * K]

        supp = []
        for c in range(SCH):
            s_c = const.tile([128, T * D], F8, name=f"supp{c}")
            supp.append(s_c)
        for half in range(2):
            cols = slice(half * 4096, (half + 1) * 4096)
            nc.gpsimd.dma_start(supp[0][:, cols], sf[0][:, cols])
            nc.gpsimd.dma_start(supp[1][:, cols], sf[1][:, cols])

        tf_t = []
        tseg_t = []
        for g in range(G):
            sg_ = const.tile([128, DCH * NW * 128], F8, name=f"tseg{g}")
            nc.gpsimd.dma_start(sg_[:], tseg[g])
            tseg_t.append(sg_)
            q_ = const.tile([128, DJ * 2 * T * 128], F8, name=f"tf{g}")
            nc.gpsimd.dma_start(q_[:], tf[g])
            tf_t.append(q_)

        # ---------------- fusion weights: fw = softmax(fus) * exp(lsc)
        # (host replicated the logits per partition, so this is pure
        # elementwise work -- the PE is never involved)
        fwc = persist.tile([128, 3], F32)
        fusw = work.tile([128, 3], F32, tag="fusw")
        nc.vector.tensor_copy(fusw[:], fus_t)
        lscw = work.tile([128, 1], F32, tag="lscw")
        nc.vector.tensor_copy(lscw[:], lsc_t)
        fmax = work.tile([128, 1], F32, tag="fmax")
        nc.vector.tensor_reduce(fmax[:], fusw[:], axis=AX.X, op=ALU.max)
        nfmax = work.tile([128, 1], F32, tag="nfmax")
        nc.vector.tensor_scalar(nfmax[:], fmax[:], -1.0, None, ALU.mult)
        fexp = work.tile([128, 3], F32, tag="fexp")
        fsum = work.tile([128, 1], F32, tag="fsum")
        nc.scalar.activation(fexp[:], fusw[:], AF.Exp, bias=nfmax[:],
                             accum_out=fsum[:])
        fdenr = work.tile([128, 1], F32, tag="fdenr")
        nc.vector.reciprocal(fdenr[:], fsum[:])
        elsc = work.tile([128, 1], F32, tag="elsc")
        nc.scalar.activation(elsc[:], lscw[:], AF.Exp)
        scl = work.tile([128, 1], F32, tag="scl")
        nc.vector.tensor_tensor(scl[:], fdenr[:], elsc[:], ALU.mult)
        nc.vector.tensor_scalar(fwc[:], fexp[:], scl[:], None, ALU.mult)


        # ---------------- prototypes (frame + segment), d-major fp8
        # protoT free layout: [j(4), o(2), ts(8), k(64)]; value = 16*nproto
        protoT = persist.tile([128, DJ * 2 * T * K], F8)
        protoT_v = protoT[:].rearrange("p (j o s k) -> p j o s k", j=DJ, o=2,
                                       s=T)
        # npsegT free layout: [dch(8), ws(3), k(64)]; value = 16*npseg
        npsegT = persist.tile([128, DCH * NW * K], F8)
        npsegT_v = npsegT[:].rearrange("p (c w k) -> p c w k", c=DCH, w=NW)
        praw = persist.tile([128, DCH * T * K], BF16)

        with tc.tile_pool(name="pscr", bufs=1) as pscr, \
             tc.tile_pool(name="psP", bufs=3, space="PSUM") as psP:
            # one-hot labels (fp8: exact 0/1)
            oh = []
            for c in range(SCH):
                oh_c = pscr.tile([128, K], F8, tag=f"oh{c}")
                nc.vector.tensor_scalar(oh_c[:], kiota_f, lab_t[c],
                                        None, ALU.is_equal)
                oh.append(oh_c)

            for dch in range(DCH):
                pp = psP.tile([128, T * K], F32, tag="pp")
                for t in range(T):
                    for c in range(SCH):
                        nc.tensor.matmul(
                            pp[:, t * K:(t + 1) * K],
                            supp[c][:, dch * (T * 128) + t * 128:
                                    dch * (T * 128) + (t + 1) * 128],
                            oh[c], start=(c == 0), stop=(c == SCH - 1))
                # keep the PE (and its clock gate) busy while the next
                # support chunk is still streaming in
                for _ in range(2):
                    nc.tensor.matmul(wps[:], wz[:, 0:128], wz[:], start=True,
                                     stop=True)
                # normalized+scaled fp8 protos (critical path); note the
                # (j, o) block of protoT is contiguous at dch*T*K
                nc.vector.tensor_tensor(
                    protoT[:, dch * T * K:(dch + 1) * T * K],
                    pp[:], rprep_t, ALU.mult)
                # raw bf16 copy for the segment prototypes (off critical path)
                nc.scalar.copy(praw[:, dch * T * K:(dch + 1) * T * K], pp[:])

            # segment prototypes: window sums over ts of praw
            praw_v = praw[:].rearrange("p (c s k) -> p c s k", c=DCH, s=T)
            ep = pscr.tile([128, DCH * 4 * K], BF16)
            ep_v = ep[:].rearrange("p (c e k) -> p c e k", c=DCH, e=4)
            for e in range(4):
                nc.vector.tensor_tensor(ep_v[:, :, e, :],
                                        praw_v[:, :, 2 * e, :],
                                        praw_v[:, :, 2 * e + 1, :], ALU.add)
            psg = pscr.tile([128, DCH * NW * K], BF16)
            psg_v = psg[:].rearrange("p (c w k) -> p c w k", c=DCH, w=NW)
            for w in range(NW):
                nc.vector.tensor_tensor(psg_v[:, :, w, :],
                                        ep_v[:, :, w, :],
                                        ep_v[:, :, w + 1, :], ALU.add)
            for dch in range(DCH):
                nc.vector.tensor_tensor(
                    npsegT[:, dch * NW * K:(dch + 1) * NW * K],
                    psg[:, dch * NW * K:(dch + 1) * NW * K],
                    rpsrep_t, ALU.mult)

        # ---------------- main loop over the 4 query slices
        obuf = persist.tile([128, 4 * G * K], F32)
        simpool = ctx.enter_context(tc.tile_pool(name="simpool", bufs=2))
        winpool = ctx.enter_context(tc.tile_pool(name="winpool", bufs=2))
        psM = ctx.enter_context(tc.tile_pool(name="psM", bufs=4, space="PSUM"))
        psS = ctx.enter_context(tc.tile_pool(name="psS", bufs=3, space="PSUM"))

        def emit_seg(g):
            """Segment GEMM + its chamfer trees (tiny). Emitted before the
            main GEMM for g>0 so all its DVE work hides under main MMs."""
            ob = g * 4 * K
            tsg = tseg_t[g][:].rearrange("p (c w q) -> p c w q", c=DCH, w=NW)
            wins = winpool.tile([128, NW * NW * K], BF16, tag="wins")
            winv = wins[:].rearrange("p (v w k) -> p v w k", v=NW, w=NW)
            for wq in range(NW):
                sp = psS.tile([128, T * K], F32, tag="sp")
                for dch in range(DCH):
                    nc.tensor.matmul(
                        sp[:, 0:NW * K], tsg[:, dch, wq, :],
                        npsegT_v[:, dch, :, :], start=(dch == 0),
                        stop=(dch == DCH - 1))
                nc.scalar.activation(
                    wins[:, wq * NW * K:(wq + 1) * NW * K],
                    sp[:, 0:NW * K], AF.Copy, scale=rqs_t[g][:, wq:wq + 1])
            # q2s: max over ws within wq, sum over wq
            sa = work.tile([128, NW * K], BF16, tag="sa")
            sav = sa[:].rearrange("p (v k) -> p v k", v=NW)
            nc.vector.tensor_tensor(sav, winv[:, :, 0, :], winv[:, :, 1, :],
                                    ALU.max)
            nc.vector.tensor_tensor(sav, sav, winv[:, :, 2, :], ALU.max)
            st = work.tile([128, K], BF16, tag="st")
            nc.vector.tensor_tensor(st[:], sa[:, 0:K], sa[:, K:2 * K],
                                    ALU.add)
            oq2s = obuf[:, ob + 3 * K:ob + 4 * K]
            nc.vector.scalar_tensor_tensor(
                oq2s, in0=sa[:, 2 * K:3 * K], scalar=-3.0, in1=st[:],
                op0=ALU.add, op1=ALU.add)
            # s2q: max over wq, sum over ws
            sm = work.tile([128, NW * K], BF16, tag="sm")
            nc.vector.tensor_tensor(sm[:], wins[:, 0:NW * K],
                                    wins[:, NW * K:2 * NW * K], ALU.max)
            nc.vector.tensor_tensor(sm[:], sm[:],
                                    wins[:, 2 * NW * K:3 * NW * K], ALU.max)
            st2 = work.tile([128, K], BF16, tag="st2")
            nc.vector.tensor_tensor(st2[:], sm[:, 0:K], sm[:, K:2 * K],
                                    ALU.add)
            os2q = obuf[:, ob + 2 * K:ob + 3 * K]
            nc.vector.scalar_tensor_tensor(
                os2q, in0=sm[:, 2 * K:3 * K], scalar=-3.0, in1=st2[:],
                op0=ALU.add, op1=ALU.add)
            # fused seg part: f1*os2q + f2*oq2s (hidden; leaves a single
            # hop for the fused output after oglo lands)
            fpart = work.tile([128, K], F32, tag="fpart")
            nc.vector.tensor_scalar(fpart[:], os2q, fwc[:, 1:2], None,
                                    ALU.mult)
            nc.vector.scalar_tensor_tensor(
                fpart[:], in0=oq2s, scalar=fwc[:, 2:3], in1=fpart[:],
                op0=ALU.mult, op1=ALU.add)
            return fpart

        def emit_main(g):
            """Main GEMM; chamfer tree halves interleave with the MM stream
            so only the tq4-7 half is exposed at the end."""
            ob = g * 4 * K
            tfg = tf_t[g][:].rearrange("p (j o t q) -> p j o t q", j=DJ, o=2,
                                       t=T)
            simcp = simpool.tile([128, T * T * K], BF16, tag="simcp")
            simv = simcp[:].rearrange("p (t s k) -> p t s k", t=T, s=T)
            for tq in range(T):
                mp = psM.tile([128, T * K], F32, tag="mp")
                for j in range(DJ):
                    nc.tensor.matmul(
                        mp[:], tfg[:, j, :, tq, :], protoT_v[:, j, :, :, :],
                        start=(j == 0), stop=(j == DJ - 1), perf_mode=DR)
                nc.scalar.activation(
                    simcp[:, tq * T * K:(tq + 1) * T * K], mp[:],
                    AF.Copy, scale=rqv_t[g][:, tq:tq + 1])
                if tq == 3:
                    # A-half trees (tq 0-3), hidden under the tq4-7 MMs
                    pm1 = work.tile([128, 1024], BF16, tag="pm1")
                    nc.vector.tensor_tensor(pm1[:], simcp[:, 0:1024],
                                            simcp[:, 1024:2048], ALU.max)
                    mrun = work.tile([128, 512], BF16, tag="mr3")
                    nc.vector.tensor_tensor(mrun[:], pm1[:, 0:512],
                                            pm1[:, 512:1024], ALU.max)
                    a1 = work.tile([128, 1024], BF16, tag="a1")
                    a1v = a1[:].rearrange("p (t s k) -> p t s k", t=4, s=4)
                    nc.vector.tensor_tensor(a1v, simv[:, 0:4, 0:4, :],
                                            simv[:, 0:4, 4:8, :], ALU.max)
                    a2 = work.tile([128, 512], BF16, tag="a2")
                    a2v = a2[:].rearrange("p (t s k) -> p t s k", t=4, s=2)
                    nc.vector.tensor_tensor(a2v, a1v[:, :, 0:2, :],
                                            a1v[:, :, 2:4, :], ALU.max)
                    am = work.tile([128, 256], BF16, tag="am")
                    amv = am[:].rearrange("p (t k) -> p t k", t=4)
                    nc.vector.tensor_tensor(amv, a2v[:, :, 0, :],
                                            a2v[:, :, 1, :], ALU.max)
                    qh = work.tile([128, 128], BF16, tag="qh")
                    nc.vector.tensor_tensor(qh[:], am[:, 0:128],
                                            am[:, 128:256], ALU.add)
                    arun = work.tile([128, K], BF16, tag="ar3")
                    nc.vector.tensor_tensor(arun[:], qh[:, 0:K], qh[:, K:128],
                                            ALU.add)
                elif tq >= 4:
                    # B-half: running reductions, each hides under its tq's
                    # MMs; after the last copy only one level is exposed.
                    # tq7's dir1 reduce runs on GpSimd so the two exposed
                    # chains overlap across engines.
                    s0 = tq * T * K
                    mnew = work.tile([128, 512], BF16, tag=f"mr{tq}")
                    nc.vector.tensor_tensor(mnew[:], mrun[:],
                                            simcp[:, s0:s0 + 512], ALU.max)
                    mrun = mnew
                    c1 = work.tile([128, 256], BF16, tag="c1")
                    nc.vector.tensor_tensor(c1[:], simcp[:, s0:s0 + 256],
                                            simcp[:, s0 + 256:s0 + 512],
                                            ALU.max)
                    c2 = work.tile([128, 128], BF16, tag="c2")
                    nc.vector.tensor_tensor(c2[:], c1[:, 0:128],
                                            c1[:, 128:256], ALU.max)
                    c3 = work.tile([128, K], BF16, tag="c3")
                    nc.vector.tensor_tensor(c3[:], c2[:, 0:K], c2[:, K:128],
                                            ALU.max)
                    anew = work.tile([128, K], BF16, tag=f"ar{tq}")
                    nc.vector.tensor_tensor(anew[:], arun[:], c3[:], ALU.add)
                    arun = anew
            # exposed tail: sum mrun over ts; -global = asum + msum - 16
            s1 = work.tile([128, 256], BF16, tag="s1")
            nc.vector.tensor_tensor(s1[:], mrun[:, 0:256], mrun[:, 256:512],
                                    ALU.add)
            s2 = work.tile([128, 128], BF16, tag="s2")
            nc.vector.tensor_tensor(s2[:], s1[:, 0:128], s1[:, 128:256],
                                    ALU.add)
            msum = work.tile([128, K], F32, tag="msum")
            nc.vector.tensor_tensor(msum[:], s2[:, 0:K], s2[:, K:128],
                                    ALU.add)
            oglo = obuf[:, ob + K:ob + 2 * K]
            nc.vector.scalar_tensor_tensor(
                oglo, in0=arun[:], scalar=-16.0, in1=msum[:],
                op0=ALU.add, op1=ALU.add)

        def emit_final(g, fpart):
            ob = g * 4 * K
            oglo = obuf[:, ob + K:ob + 2 * K]
            ofus = obuf[:, ob:ob + K]
            nc.vector.scalar_tensor_tensor(
                ofus, in0=oglo, scalar=fwc[:, 0:1], in1=fpart[:],
                op0=ALU.mult, op1=ALU.add)
            nc.gpsimd.dma_start(oall[:, ob:ob + 4 * K],
                                obuf[:, ob:ob + 4 * K])

        for g in range(G):
            if g == 0:
                emit_main(g)
                fpart = emit_seg(g)
            else:
                fpart = emit_seg(g)
                emit_main(g)
            emit_final(g, fpart)

    nc.compile()
    return nc


_NC_CACHE = None


def _get_nc():
    global _NC_CACHE
    if _NC_CACHE is None:
        _NC_CACHE = build_nc()
    return _NC_CACHE


# ------------------------------------------------------------------ host side
def build_in_maps(support_features, target_features, support_labels,
                  logit_scale, fusion_logits):
    support_features = np.asarray(support_features, dtype=np.float32)
    target_features = np.asarray(target_features, dtype=np.float32)
    support_labels = np.asarray(support_labels, dtype=np.int32)
    logit_scale = np.asarray(logit_scale, dtype=np.float32)
    fusion_logits = np.asarray(fusion_logits, dtype=np.float32)

    # ---- support: fp8 cast, s-major [c, s, (dch, t, d128)]
    s8 = support_features.astype(NP_F8)                    # [256, 8, 1024]
    sf_h = np.ascontiguousarray(
        s8.reshape(SCH, 128, T, DCH, 128).transpose(0, 1, 3, 2, 4)
    ).reshape(SCH, 128, T * D)

    # ---- replicated proto norm rows (from the same fp8 values the device
    # accumulates, so they match the on-device prototype sums)
    s8f = s8.astype(np.float32)
    proto_sum = np.zeros((K, T, D), np.float32)
    np.add.at(proto_sum, support_labels % K, s8f)
    rp = FSCALE / np.sqrt((proto_sum * proto_sum).sum(-1))   # [K, T]
    segp = np.stack([proto_sum[:, s:e].sum(1) for s, e in WINDOWS], 1)
    rps = FSCALE / np.sqrt((segp * segp).sum(-1))            # [K, NW]
    smallb_h = np.ascontiguousarray(np.broadcast_to(
        np.concatenate([rp.T.reshape(-1), rps.T.reshape(-1)]
                       ).reshape(1, T * K + NW * K),
        (128, T * K + NW * K))).astype(NP_BF16)

    fus_rep = np.broadcast_to(fusion_logits.reshape(1, 3), (128, 3))
    lsc_rep = np.broadcast_to(logit_scale.reshape(1, 1), (128, 1))
    labcols = support_labels.astype(np.float32).reshape(SCH, 128).T  # [128,2]
    kio = np.broadcast_to(np.arange(K, dtype=np.float32).reshape(1, K),
                          (128, K))

    in_maps = []
    for c in range(NCORES):
        x = target_features[c * QPC:(c + 1) * QPC]           # [512, 8, 1024]
        x8 = x.astype(NP_F8)
        x8f = x8.astype(np.float32)
        # d-major per-slice layout [g, p, j, o, t, q]
        tf_h = np.ascontiguousarray(
            x8.reshape(G, 128, T, DJ, 2, 128).transpose(0, 5, 3, 4, 2, 1)
        ).reshape(G, 128, DJ * 2 * T * 128)
        rqv_h = (1.0 / (FSCALE * np.sqrt((x8f * x8f).sum(-1)))
                 ).reshape(G, 128, T)
        # window-summed segments, re-cast to fp8, d-major [g, p, dch, w, q]
        segf = np.stack([x8f[:, s:e].sum(1) for s, e in WINDOWS], 1)
        seg8 = segf.astype(NP_F8)
        seg8f = seg8.astype(np.float32)
        tseg_h = np.ascontiguousarray(
            seg8.reshape(G, 128, NW, DCH, 128).transpose(0, 4, 3, 2, 1)
        ).reshape(G, 128, DCH * NW * 128)
        rqs_h = (1.0 / (FSCALE * np.sqrt((seg8f * seg8f).sum(-1)))
                 ).reshape(G, 128, NW)
        # packed f32 constants:
        # [lab0, lab1, rqv(32), rqs(12), kiota(64), fus(3), lsc(1)]
        smallf_h = np.ascontiguousarray(np.concatenate(
            [labcols,
             rqv_h.transpose(1, 0, 2).reshape(128, G * T),
             rqs_h.transpose(1, 0, 2).reshape(128, G * NW),
             kio, fus_rep, lsc_rep], axis=1).astype(np.float32))
        in_maps.append({
            "tf": tf_h, "tseg": tseg_h, "sf": sf_h,
            "smallf": smallf_h, "smallb": smallb_h,
        })
    return in_maps


def kernel(support_features, target_features, support_labels, logit_scale,
           fusion_logits):
    in_maps = build_in_maps(support_features, target_features, support_labels,
                            logit_scale, fusion_logits)
    nc = _get_nc()
    res = run_bass_kernel_spmd(nc, in_maps, core_ids=list(range(NCORES)))

    outs = []
    for w in range(4):
        parts = []
        for c in range(NCORES):
            blob = np.asarray(res.results[c]["oall"]).reshape(128, G, 4, K)
            # [p, g, which, k] -> queries q = g*128 + p
            parts.append(blob[:, :, w].transpose(1, 0, 2).reshape(QPC, K))
        outs.append(np.concatenate(parts, axis=0).astype(np.float32))
    return tuple(outs)


if __name__ == "__main__":
    rng = np.random.default_rng(0)
    ins = {
        "support_features": rng.standard_normal((S, T, D), dtype=np.float32),
        "target_features": rng.standard_normal((Q, T, D), dtype=np.float32),
        "support_labels": (np.arange(S) % K).astype(np.int32),
        "logit_scale": np.float32(0.0),
        "fusion_logits": np.zeros(3, np.float32),
    }
    outs = kernel(**ins)
    for o in outs:
        print(o.shape, o.dtype, float(o.mean()))



# revision 3
# speedup vs baseline: 1.1282x; 1.1282x over previous
"""Trainium2 Bass kernel for few-shot video retrieval (bidirectional chamfer
distance to class prototypes, global frame-level + segment-level, fused).

Contract: kernel(**inputs) takes the FULL unsharded inputs (numpy) and returns
the full outputs (tuple of 4 [4096, 64] float32 arrays), matching reference().

Sharding: data-parallel over the query axis across 8 NeuronCores; prototypes
(computed on host, like the norm factors) replicated. Gather + fusion on host.

Device-side algorithm per core (512 queries = 4 slices of 128):
  - host pre-normalizes every query frame (x64) and every prototype frame
    (x16) in f32, then casts to fp8 e4m3 -> all PSUM results are 1024*sim
    with a single constant drain scale; no per-(q,t) norm factors on device
  - main sims GEMM: queries stationary (d-major), protoT moving, fp8
    DoubleRow (256-deep contraction), output [q, (k, ts)] with ts innermost
  - chamfer: dir0 (max over ts) = two grouped tensor_reduce ops (contiguous
    innermost axis, 2x bf16); dir1 (max over tq) = pairwise bf16 max TTs that
    pipeline with the PSUM drains; sums via strided reduces
  - segments: 3 separate GEMM groups (one per support window v), stationary
    = seg prototypes [d, k] so outputs land k-major on partitions 0-63;
    chamfer trees split between DVE and GpSimd
  - fusion softmax/exp + final gather/transpose on host
"""

import sys

sys.path.insert(0, "/opt/trn_rl_repo")

import numpy as np
import ml_dtypes
from contextlib import ExitStack

import concourse.bass as bass
import concourse.bacc as bacc
import concourse.tile as tile
from concourse import mybir
from concourse.bass_utils import run_bass_kernel_spmd

# ---------------------------------------------------------------- problem dims
S, Q, T, D = 256, 4096, 8, 1024
K = 64                      # classes
NCORES = 8
QPC = Q // NCORES           # 512 queries per core
G = QPC // 128              # 4 query-slices of 128 per core
DJ = 4                      # 4 DoubleRow chunks (256-deep)
NW = 3                      # segment windows
WINDOWS = ((0, 4), (2, 6), (4, 8))
QSC = 64.0                  # query fp8 scale (host-normalized frames)
PSC = 16.0                  # prototype fp8 scale
ISC = 1.0 / (QSC * PSC)     # drain scale: PSUM value = 1024 * sim

F32 = mybir.dt.float32
BF16 = mybir.dt.bfloat16
F8 = mybir.dt.float8e4
AF = mybir.ActivationFunctionType
ALU = mybir.AluOpType
AX = mybir.AxisListType
DR = mybir.MatmulPerfMode.DoubleRow

NP_F8 = ml_dtypes.float8_e4m3


# ---------------------------------------------------------------- bass kernel
def build_nc():
    nc = bacc.Bacc("TRN2", target_bir_lowering=False, debug=False,
                   num_devices=NCORES)

    # d-major normalized queries: [g][p(d%128)][(j, o, t, q)]
    tf = nc.dram_tensor("tf", [G, 128, DJ * 2 * T * 128], F8,
                        kind="ExternalInput")
    # d-major normalized query segments: [g][p][(j, o, w, q)]
    tsg = nc.dram_tensor("tsg", [G, 128, DJ * 2 * NW * 128], F8,
                         kind="ExternalInput")
    # d-major normalized frame prototypes: [p][(j, o, k, s)]
    pT = nc.dram_tensor("pT", [128, DJ * 2 * K * T], F8, kind="ExternalInput")
    # d-major normalized segment prototypes: [p][(j, o, v, k)]
    sT = nc.dram_tensor("sT", [128, DJ * 2 * NW * K], F8,
                        kind="ExternalInput")

    # outputs: -global_dist q-major; -(s2q|q2s) k-major
    oglo = nc.dram_tensor("oglo", [128, G * K], F32, kind="ExternalOutput")
    oseg = nc.dram_tensor("oseg", [64, G * 2 * 128], F32,
                          kind="ExternalOutput")

    NWARM = 16

    with tile.TileContext(nc) as tc, ExitStack() as ctx:
        const = ctx.enter_context(tc.tile_pool(name="const", bufs=1))
        persist = ctx.enter_context(tc.tile_pool(name="persist", bufs=1))
        simpool = ctx.enter_context(tc.tile_pool(name="simpool", bufs=2))
        segpool = ctx.enter_context(tc.tile_pool(name="segpool", bufs=2))
        work = ctx.enter_context(tc.tile_pool(name="work", bufs=2))

        # ---------------- PE warmup burst (HAM ramp while inputs stream in);
        # memset is the first gpsimd instruction so it lands ~instantly
        wz = const.tile([128, 256], F8)
        nc.gpsimd.memset(wz[:], 0)
        with tc.tile_pool(name="psW", bufs=1, space="PSUM") as psW:
            wps = psW.tile([128, 256], F32)
            for _ in range(NWARM):
                nc.tensor.matmul(wps[:], wz[:, 0:128], wz[:], start=True,
                                 stop=True)

        # ---------------- input DMAs (issue order = priority order)
        pT_t = const.tile([128, DJ * 2 * K * T], F8)
        for h in range(2):
            cols = slice(h * 2048, (h + 1) * 2048)
            nc.gpsimd.dma_start(pT_t[:, cols], pT[:, cols])
        tf_t = []
        tsg_t = []
        for g in range(G):
            tf_t.append(const.tile([128, DJ * 2 * T * 128], F8,
                                   name=f"tf{g}"))
            tsg_t.append(const.tile([128, DJ * 2 * NW * 128], F8,
                                    name=f"tsg{g}"))
        for h in range(2):
            cols = slice(h * 4096, (h + 1) * 4096)
            nc.gpsimd.dma_start(tf_t[0][:, cols], tf[0][:, cols])
        sT_t = const.tile([128, DJ * 2 * NW * K], F8)
        nc.gpsimd.dma_start(sT_t[:], sT[:])
        nc.gpsimd.dma_start(tsg_t[0][:], tsg[0])
        for h in range(2):
            cols = slice(h * 4096, (h + 1) * 4096)
            nc.gpsimd.dma_start(tf_t[1][:, cols], tf[1][:, cols])
        nc.gpsimd.dma_start(tsg_t[1][:], tsg[1])
        nc.gpsimd.dma_start(tsg_t[2][:], tsg[2])
        for h in range(2):
            cols = slice(h * 4096, (h + 1) * 4096)
            nc.gpsimd.dma_start(tf_t[2][:, cols], tf[2][:, cols])
        nc.gpsimd.dma_start(tsg_t[3][:], tsg[3])
        for h in range(2):
            cols = slice(h * 4096, (h + 1) * 4096)
            nc.gpsimd.dma_start(tf_t[3][:, cols], tf[3][:, cols])

        pT_v = pT_t[:].rearrange("p (j o k s) -> p j o k s", j=DJ, o=2, k=K)
        sT_v = sT_t[:].rearrange("p (j o v k) -> p j o v k", j=DJ, o=2, v=NW)

        obuf_glo = persist.tile([128, G * K], F32)
        obuf_seg = persist.tile([64, G * 2 * 128], F32)

        psM = ctx.enter_context(tc.tile_pool(name="psM", bufs=3,
                                             space="PSUM"))
        psS = ctx.enter_context(tc.tile_pool(name="psS", bufs=1,
                                             space="PSUM"))

        def emit_main(g):
            tf_v = tf_t[g][:].rearrange("p (j o t q) -> p j o t q", j=DJ,
                                        o=2, t=T)
            simcp = simpool.tile([128, T * K * T], BF16, tag="simcp")
            pmax = simpool.tile([128, 4 * K * T], BF16, tag="pmax")
            Aall = work.tile([128, 2 * 4 * K], BF16, tag="Aall")
            simv = simcp[:].rearrange("p (t k s) -> p t k s", t=T, k=K)
            for tq in range(T):
                mp = psM.tile([128, K * T], F32, tag="mp")
                for j in range(DJ):
                    nc.tensor.matmul(mp[:], tf_v[:, j, :, tq, :],
                                     pT_v[:, j], start=(j == 0),
                                     stop=(j == DJ - 1), perf_mode=DR)
                nc.scalar.activation(simcp[:, tq * 512:(tq + 1) * 512],
                                     mp[:], AF.Copy, scale=ISC)
                if tq % 2 == 1:
                    i = tq // 2
                    nc.vector.tensor_tensor(
                        pmax[:, i * 512:(i + 1) * 512],
                        simcp[:, (tq - 1) * 512:tq * 512],
                        simcp[:, tq * 512:(tq + 1) * 512], ALU.max)
                if tq == 3:
                    # dir1 half-merge + dir0 first half (hide under tq4-7)
                    T1 = work.tile([128, 512], BF16, tag="T1")
                    nc.vector.tensor_tensor(T1[:], pmax[:, 0:512],
                                            pmax[:, 512:1024], ALU.max)
                    nc.vector.tensor_reduce(Aall[:, 0:256], simv[:, 0:4],
                                            axis=AX.X, op=ALU.max)
            T2 = work.tile([128, 512], BF16, tag="T2")
            nc.vector.tensor_tensor(T2[:], pmax[:, 1024:1536],
                                    pmax[:, 1536:2048], ALU.max)
            Rm = work.tile([128, 512], BF16, tag="Rm")
            nc.vector.tensor_tensor(Rm[:], T1[:], T2[:], ALU.max)
            msum = work.tile([128, K], F32, tag="msum")
            nc.vector.tensor_reduce(msum[:],
                                    Rm[:].rearrange("p (k s) -> p k s", k=K),
                                    axis=AX.X, op=ALU.add)
            nc.vector.tensor_reduce(Aall[:, 256:512], simv[:, 4:8],
                                    axis=AX.X, op=ALU.max)
            asum = work.tile([128, K], F32, tag="asum")
            nc.vector.tensor_reduce(
                asum[:],
                Aall[:].rearrange("p (h t k) -> p k h t", h=2, t=4),
                axis=AX.XY, op=ALU.add)
            nc.vector.scalar_tensor_tensor(
                obuf_glo[:, g * K:(g + 1) * K], in0=asum[:], scalar=-16.0,
                in1=msum[:], op0=ALU.add, op1=ALU.add)
            nc.gpsimd.dma_start(oglo[:, g * K:(g + 1) * K],
                                obuf_glo[:, g * K:(g + 1) * K])

        def emit_seg(g):
            tsg_v = tsg_t[g][:].rearrange("p (j o w q) -> p j o w q", j=DJ,
                                          o=2, w=NW)
            segs = []
            for v in range(NW):
                sp = psS.tile([64, NW * 128], F32, tag=f"sv{v}")
                for j in range(DJ):
                    nc.tensor.matmul(sp[:], sT_v[:, j, :, v, :],
                                     tsg_v[:, j], start=(j == 0),
                                     stop=(j == DJ - 1), perf_mode=DR)
                sg = segpool.tile([64, NW * 128], BF16, tag=f"sg{v}")
                nc.scalar.activation(sg[:], sp[:], AF.Copy, scale=ISC)
                segs.append(sg)
            # q2s = sum_w max_v  (max over v on DVE: wide 384 ops)
            m01 = work.tile([64, NW * 128], BF16, tag="m01")
            nc.vector.tensor_tensor(m01[:], segs[0][:], segs[1][:], ALU.max)
            m012 = work.tile([64, NW * 128], BF16, tag="m012")
            nc.vector.tensor_tensor(m012[:], m01[:], segs[2][:], ALU.max)
            ws = work.tile([64, 128], BF16, tag="ws")
            nc.vector.tensor_tensor(ws[:], m012[:, 0:128], m012[:, 128:256],
                                    ALU.add)
            nc.vector.scalar_tensor_tensor(
                obuf_seg[:, g * 256 + 128:g * 256 + 256], in0=ws[:],
                scalar=-3.0, in1=m012[:, 256:384], op0=ALU.add, op1=ALU.add)
            # s2q = sum_v max_w
            wv = []
            for v in range(NW):
                w1 = work.tile([64, 128], BF16, tag=f"w1{v}")
                nc.vector.tensor_tensor(w1[:], segs[v][:, 0:128],
                                        segs[v][:, 128:256], ALU.max)
                w2 = work.tile([64, 128], BF16, tag=f"w2{v}")
                nc.vector.tensor_tensor(w2[:], w1[:], segs[v][:, 256:384],
                                        ALU.max)
                wv.append(w2)
            vs = work.tile([64, 128], BF16, tag="vs")
            nc.vector.tensor_tensor(vs[:], wv[0][:], wv[1][:], ALU.add)
            nc.vector.scalar_tensor_tensor(
                obuf_seg[:, g * 256:g * 256 + 128], in0=vs[:], scalar=-3.0,
                in1=wv[2][:], op0=ALU.add, op1=ALU.add)
            nc.gpsimd.dma_start(oseg[:, g * 256:(g + 1) * 256],
                                obuf_seg[:, g * 256:(g + 1) * 256])

        for g in range(G):
            if g == 0:
                emit_main(g)
                emit_seg(g)
            else:
                emit_seg(g)
                emit_main(g)

    nc.compile()
    return nc


_NC_CACHE = None


def _get_nc():
    global _NC_CACHE
    if _NC_CACHE is None:
        _NC_CACHE = build_nc()
    return _NC_CACHE


# ------------------------------------------------------------------ host side
def _norm8(x, scale):
    n = np.sqrt((x * x).sum(-1, keepdims=True))
    n = np.maximum(n, 1e-12)
    return (scale * x / n).astype(NP_F8)


def build_in_maps(support_features, target_features, support_labels,
                  logit_scale, fusion_logits):
    support_features = np.asarray(support_features, dtype=np.float32)
    target_features = np.asarray(target_features, dtype=np.float32)
    support_labels = np.asarray(support_labels, dtype=np.int32)

    # ---- prototypes (exact f32 scatter-mean, normalized, x16, fp8)
    proto = np.zeros((K, T, D), np.float32)
    cnt = np.zeros((K,), np.float32)
    np.add.at(proto, support_labels % K, support_features)
    np.add.at(cnt, support_labels % K, 1.0)
    proto /= cnt[:, None, None]
    p8 = _norm8(proto, PSC)                                   # [K, T, D]
    segp = np.stack([proto[:, s:e].sum(1) for s, e in WINDOWS], 1)
    sp8 = _norm8(segp, PSC)                                   # [K, NW, D]

    # pT: [p][(j,o,k,s)]
    pT_h = np.ascontiguousarray(
        p8.reshape(K, T, DJ, 2, 128).transpose(4, 2, 3, 0, 1)
    ).reshape(128, DJ * 2 * K * T)
    # sT: [p][(j,o,v,k)]
    sT_h = np.ascontiguousarray(
        sp8.reshape(K, NW, DJ, 2, 128).transpose(4, 2, 3, 1, 0)
    ).reshape(128, DJ * 2 * NW * K)

    # ---- queries: normalized x64, fp8, d-major
    q8 = _norm8(target_features, QSC)                         # [Q, T, D]
    segq = np.stack([target_features[:, s:e].sum(1) for s, e in WINDOWS], 1)
    sq8 = _norm8(segq, QSC)                                   # [Q, NW, D]

    in_maps = []
    for c in range(NCORES):
        x8 = q8[c * QPC:(c + 1) * QPC]
        tf_h = np.ascontiguousarray(
            x8.reshape(G, 128, T, DJ, 2, 128).transpose(0, 5, 3, 4, 2, 1)
        ).reshape(G, 128, DJ * 2 * T * 128)
        s8 = sq8[c * QPC:(c + 1) * QPC]
        tsg_h = np.ascontiguousarray(
            s8.reshape(G, 128, NW, DJ, 2, 128).transpose(0, 5, 3, 4, 2, 1)
        ).reshape(G, 128, DJ * 2 * NW * 128)
        in_maps.append({"tf": tf_h, "tsg": tsg_h, "pT": pT_h, "sT": sT_h})
    return in_maps


def kernel(support_features, target_features, support_labels, logit_scale,
           fusion_logits):
    logit_scale = np.asarray(logit_scale, dtype=np.float32)
    fusion_logits = np.asarray(fusion_logits, dtype=np.float32)
    in_maps = build_in_maps(support_features, target_features, support_labels,
                            logit_scale, fusion_logits)
    nc = _get_nc()
    res = run_bass_kernel_spmd(nc, in_maps, core_ids=list(range(NCORES)))

    glo = np.empty((Q, K), np.float32)
    s2q = np.empty((Q, K), np.float32)
    q2s = np.empty((Q, K), np.float32)
    for c in range(NCORES):
        og = np.asarray(res.results[c]["oglo"]).reshape(128, G, K)
        glo[c * QPC:(c + 1) * QPC] = og.transpose(1, 0, 2).reshape(QPC, K)
        os_ = np.asarray(res.results[c]["oseg"]).reshape(64, G, 2, 128)
        # [k, g, which, q] -> [g, q, k]
        s2q[c * QPC:(c + 1) * QPC] = os_[:, :, 0].transpose(1, 2, 0).reshape(
            QPC, K)
        q2s[c * QPC:(c + 1) * QPC] = os_[:, :, 1].transpose(1, 2, 0).reshape(
            QPC, K)

    e = np.exp(fusion_logits - fusion_logits.max())
    fw = (e / e.sum()) * np.exp(logit_scale)
    fused = fw[0] * glo + fw[1] * s2q + fw[2] * q2s
    return (fused.astype(np.float32), glo, s2q, q2s)


if __name__ == "__main__":
    rng = np.random.default_rng(0)
    ins = {
        "support_features": rng.standard_normal((S, T, D), dtype=np.float32),
        "target_features": rng.standard_normal((Q, T, D), dtype=np.float32),
        "support_labels": (np.arange(S) % K).astype(np.int32),
        "logit_scale": np.float32(0.0),
        "fusion_logits": np.zeros(3, np.float32),
    }
    outs = kernel(**ins)
    for o in outs:
        print(o.shape, o.dtype, float(o.mean()))


# revision 5
# speedup vs baseline: 1.2649x; 1.1212x over previous
"""Trainium2 Bass kernel for few-shot video retrieval (bidirectional chamfer
distance to class prototypes, global frame-level + segment-level, fused).

Contract: kernel(**inputs) takes the FULL unsharded inputs (numpy) and returns
the full outputs (tuple of 4 [4096, 64] float32 arrays), matching reference().

Sharding: data-parallel over the query axis across 8 NeuronCores; prototypes
(computed on host, like the norm factors) replicated. Gather + fusion on host.

Device-side algorithm per core (512 queries = 4 slices of 128):
  - host pre-normalizes every query frame (x64) and every prototype frame
    (x16) in f32, then casts to fp8 e4m3 -> all PSUM results are 1024*sim
    with a single constant drain scale; no per-(q,t) norm factors on device
  - main sims GEMM: queries stationary (d-major), protoT moving, fp8
    DoubleRow (256-deep contraction), output [q, (k, ts)] with ts innermost
  - chamfer: dir0 (max over ts) = two grouped tensor_reduce ops (contiguous
    innermost axis, 2x bf16); dir1 (max over tq) = pairwise bf16 max TTs that
    pipeline with the PSUM drains; sums via strided reduces
  - segments: 3 separate GEMM groups (one per support window v), stationary
    = seg prototypes [d, k] so outputs land k-major on partitions 0-63;
    chamfer trees split between DVE and GpSimd
  - fusion softmax/exp + final gather/transpose on host
"""

import sys

sys.path.insert(0, "/opt/trn_rl_repo")

import numpy as np
import ml_dtypes
from contextlib import ExitStack

import concourse.bass as bass
import concourse.bacc as bacc
import concourse.tile as tile
from concourse import mybir
from concourse.bass_utils import run_bass_kernel_spmd

# ---------------------------------------------------------------- problem dims
S, Q, T, D = 256, 4096, 8, 1024
K = 64                      # classes
NCORES = 8
QPC = Q // NCORES           # 512 queries per core
G = QPC // 128              # 4 query-slices of 128 per core
DJ = 4                      # 4 DoubleRow chunks (256-deep)
NW = 3                      # segment windows
WINDOWS = ((0, 4), (2, 6), (4, 8))
QSC = 64.0                  # query fp8 scale (host-normalized frames)
PSC = 16.0                  # prototype fp8 scale
ISC = 1.0 / (QSC * PSC)     # drain scale: PSUM value = 1024 * sim

F32 = mybir.dt.float32
BF16 = mybir.dt.bfloat16
F8 = mybir.dt.float8e4
AF = mybir.ActivationFunctionType
ALU = mybir.AluOpType
AX = mybir.AxisListType
DR = mybir.MatmulPerfMode.DoubleRow

NP_F8 = ml_dtypes.float8_e4m3


# ---------------------------------------------------------------- bass kernel
def build_nc():
    nc = bacc.Bacc("TRN2", target_bir_lowering=False, debug=False,
                   num_devices=NCORES)

    # d-major normalized queries: [g][p(d%128)][(j, o, t, q)]
    tf = nc.dram_tensor("tf", [G, 128, DJ * 2 * T * 128], F8,
                        kind="ExternalInput")
    # d-major normalized query segments: [g][p][(j, o, w, q)]
    tsg = nc.dram_tensor("tsg", [G, 128, DJ * 2 * NW * 128], F8,
                         kind="ExternalInput")
    # d-major normalized frame prototypes: [p][(j, o, k, s)]
    pT = nc.dram_tensor("pT", [128, DJ * 2 * K * T], F8, kind="ExternalInput")
    # d-major normalized segment prototypes: [p][(j, o, v, k)]
    sT = nc.dram_tensor("sT", [128, DJ * 2 * NW * K], F8,
                        kind="ExternalInput")

    # outputs: -global_dist q-major; -(s2q|q2s) k-major
    oglo = nc.dram_tensor("oglo", [128, G * K], F32, kind="ExternalOutput")
    oseg = nc.dram_tensor("oseg", [64, G * 2 * 128], F32,
                          kind="ExternalOutput")

    NWARM = 16

    with tile.TileContext(nc) as tc, ExitStack() as ctx:
        const = ctx.enter_context(tc.tile_pool(name="const", bufs=1))
        persist = ctx.enter_context(tc.tile_pool(name="persist", bufs=1))
        simpool = ctx.enter_context(tc.tile_pool(name="simpool", bufs=2))
        segpool = ctx.enter_context(tc.tile_pool(name="segpool", bufs=2))
        work = ctx.enter_context(tc.tile_pool(name="work", bufs=2))

        # ---------------- PE warmup burst (HAM ramp while inputs stream in);
        # memset is the first gpsimd instruction so it lands ~instantly
        wz = const.tile([128, 256], F8)
        nc.gpsimd.memset(wz[:], 0)
        with tc.tile_pool(name="psW", bufs=1, space="PSUM") as psW:
            wps = psW.tile([128, 256], F32)
            for _ in range(NWARM):
                nc.tensor.matmul(wps[:], wz[:, 0:128], wz[:], start=True,
                                 stop=True)

        # ---------------- input DMAs (issue order = priority order)
        pT_t = const.tile([128, DJ * 2 * K * T], F8)
        for h in range(2):
            cols = slice(h * 2048, (h + 1) * 2048)
            nc.gpsimd.dma_start(pT_t[:, cols], pT[:, cols])
        tf_t = []
        tsg_t = []
        for g in range(G):
            tf_t.append(const.tile([128, DJ * 2 * T * 128], F8,
                                   name=f"tf{g}"))
            tsg_t.append(const.tile([128, DJ * 2 * NW * 128], F8,
                                    name=f"tsg{g}"))
        for h in range(2):
            cols = slice(h * 4096, (h + 1) * 4096)
            nc.gpsimd.dma_start(tf_t[0][:, cols], tf[0][:, cols])
        sT_t = const.tile([128, DJ * 2 * NW * K], F8)
        nc.gpsimd.dma_start(sT_t[:], sT[:])
        nc.gpsimd.dma_start(tsg_t[0][:], tsg[0])
        for h in range(2):
            cols = slice(h * 4096, (h + 1) * 4096)
            nc.gpsimd.dma_start(tf_t[1][:, cols], tf[1][:, cols])
        nc.gpsimd.dma_start(tsg_t[1][:], tsg[1])
        nc.gpsimd.dma_start(tsg_t[2][:], tsg[2])
        for h in range(2):
            cols = slice(h * 4096, (h + 1) * 4096)
            nc.gpsimd.dma_start(tf_t[2][:, cols], tf[2][:, cols])
        nc.gpsimd.dma_start(tsg_t[3][:], tsg[3])
        for h in range(2):
            cols = slice(h * 4096, (h + 1) * 4096)
            nc.gpsimd.dma_start(tf_t[3][:, cols], tf[3][:, cols])

        pT_v = pT_t[:].rearrange("p (j o k s) -> p j o k s", j=DJ, o=2, k=K)
        sT_v = sT_t[:].rearrange("p (j o v k) -> p j o v k", j=DJ, o=2, v=NW)

        obuf_glo = persist.tile([128, G * K], F32)
        obuf_seg = persist.tile([64, G * 2 * 128], F32)

        psM = ctx.enter_context(tc.tile_pool(name="psM", bufs=3,
                                             space="PSUM"))
        psS = ctx.enter_context(tc.tile_pool(name="psS", bufs=1,
                                             space="PSUM"))

        def emit_main(g):
            tf_v = tf_t[g][:].rearrange("p (j o t q) -> p j o t q", j=DJ,
                                        o=2, t=T)
            simcp = simpool.tile([128, T * K * T], BF16, tag="simcp")
            pmax = simpool.tile([128, 4 * K * T], BF16, tag="pmax")
            Lh = simpool.tile([128, 2 * 4 * K * 4], BF16, tag="Lh")
            for tq in range(T):
                mp = psM.tile([128, K * T], F32, tag="mp")
                for j in range(DJ):
                    nc.tensor.matmul(mp[:], tf_v[:, j, :, tq, :],
                                     pT_v[:, j], start=(j == 0),
                                     stop=(j == DJ - 1), perf_mode=DR)
                nc.scalar.activation(simcp[:, tq * 512:(tq + 1) * 512],
                                     mp[:], AF.Copy, scale=ISC)
                if tq % 2 == 1:
                    i = tq // 2
                    nc.vector.tensor_tensor(
                        pmax[:, i * 512:(i + 1) * 512],
                        simcp[:, (tq - 1) * 512:tq * 512],
                        simcp[:, tq * 512:(tq + 1) * 512], ALU.max)
                if tq == 3 or tq == 7:
                    # dir1 half-merge + dir0 s-halving L1 (hide under MMs)
                    h = tq // 4
                    Th = work.tile([128, 512], BF16, tag=f"T{h}")
                    nc.vector.tensor_tensor(
                        Th[:], pmax[:, h * 1024:h * 1024 + 512],
                        pmax[:, h * 1024 + 512:h * 1024 + 1024], ALU.max)
                    if h == 0:
                        T1 = Th
                    else:
                        T2 = Th
                    sh = simcp[:, h * 2048:(h + 1) * 2048].rearrange(
                        "p (tk s) -> p tk s", s=T)
                    nc.vector.tensor_tensor(
                        Lh[:, h * 1024:(h + 1) * 1024].rearrange(
                            "p (tk s) -> p tk s", s=4),
                        sh[:, :, 0:4], sh[:, :, 4:8], ALU.max)
            Rm = work.tile([128, 512], BF16, tag="Rm")
            nc.vector.tensor_tensor(Rm[:], T1[:], T2[:], ALU.max)
            msum = work.tile([128, K], F32, tag="msum")
            nc.vector.tensor_reduce(msum[:],
                                    Rm[:].rearrange("p (k s) -> p k s", k=K),
                                    axis=AX.X, op=ALU.add)
            # dir0 L2/L3: (h,t,k,s4) -> (h,t,k)
            L2 = work.tile([128, 1024], BF16, tag="L2")
            lhv = Lh[:].rearrange("p (tk s) -> p tk s", s=4)
            nc.vector.tensor_tensor(
                L2[:].rearrange("p (tk s) -> p tk s", s=2),
                lhv[:, :, 0:2], lhv[:, :, 2:4], ALU.max)
            Am = work.tile([128, 512], BF16, tag="Am")
            l2v = L2[:].rearrange("p (tk s) -> p tk s", s=2)
            nc.vector.tensor_tensor(Am[:].rearrange("p (tk s) -> p tk s",
                                                    s=1),
                                    l2v[:, :, 0:1], l2v[:, :, 1:2], ALU.max)
            # asum tree over t: Am layout (h2, t4, k64)
            h1 = work.tile([128, 256], BF16, tag="h1")
            nc.vector.tensor_tensor(h1[:], Am[:, 0:256], Am[:, 256:512],
                                    ALU.add)
            h2 = work.tile([128, 128], BF16, tag="h2")
            nc.vector.tensor_tensor(h2[:], h1[:, 0:128], h1[:, 128:256],
                                    ALU.add)
            asum = work.tile([128, K], F32, tag="asum")
            nc.vector.tensor_tensor(asum[:], h2[:, 0:64], h2[:, 64:128],
                                    ALU.add)
            nc.vector.scalar_tensor_tensor(
                obuf_glo[:, g * K:(g + 1) * K], in0=asum[:], scalar=-16.0,
                in1=msum[:], op0=ALU.add, op1=ALU.add)
            nc.gpsimd.dma_start(oglo[:, g * K:(g + 1) * K],
                                obuf_glo[:, g * K:(g + 1) * K])

        def emit_seg(g):
            tsg_v = tsg_t[g][:].rearrange("p (j o w q) -> p j o w q", j=DJ,
                                          o=2, w=NW)
            segs = segpool.tile([64, NW * NW * 128], BF16, tag="segs")
            for v in range(NW):
                sp = psS.tile([64, NW * 128], F32, tag=f"sv{v}")
                for j in range(DJ):
                    nc.tensor.matmul(sp[:], sT_v[:, j, :, v, :],
                                     tsg_v[:, j], start=(j == 0),
                                     stop=(j == DJ - 1), perf_mode=DR)
                nc.scalar.activation(segs[:, v * 384:(v + 1) * 384], sp[:],
                                     AF.Copy, scale=ISC)
            # q2s = sum_w max_v  (contiguous 384-wide maxes over v-slices)
            m01 = work.tile([64, NW * 128], BF16, tag="m01")
            nc.vector.tensor_tensor(m01[:], segs[:, 0:384], segs[:, 384:768],
                                    ALU.max)
            m012 = work.tile([64, NW * 128], BF16, tag="m012")
            nc.vector.tensor_tensor(m012[:], m01[:], segs[:, 768:1152],
                                    ALU.max)
            ws = work.tile([64, 128], BF16, tag="ws")
            nc.vector.tensor_tensor(ws[:], m012[:, 0:128], m012[:, 128:256],
                                    ALU.add)
            nc.vector.scalar_tensor_tensor(
                obuf_seg[:, g * 256 + 128:g * 256 + 256], in0=ws[:],
                scalar=-3.0, in1=m012[:, 256:384], op0=ALU.add, op1=ALU.add)
            # s2q = sum_v max_w  (strided views over w, v in the free dim)
            vv = segs[:].rearrange("p (v w q) -> p v w q", v=NW, w=NW)
            W1 = work.tile([64, NW * 128], BF16, tag="W1")
            w1v = W1[:].rearrange("p (v q) -> p v q", v=NW)
            nc.vector.tensor_tensor(w1v, vv[:, :, 0, :], vv[:, :, 1, :],
                                    ALU.max)
            Wm = work.tile([64, NW * 128], BF16, tag="Wm")
            wmv = Wm[:].rearrange("p (v q) -> p v q", v=NW)
            nc.vector.tensor_tensor(wmv, w1v, vv[:, :, 2, :], ALU.max)
            vs = work.tile([64, 128], BF16, tag="vs")
            nc.vector.tensor_tensor(vs[:], Wm[:, 0:128], Wm[:, 128:256],
                                    ALU.add)
            nc.vector.scalar_tensor_tensor(
                obuf_seg[:, g * 256:g * 256 + 128], in0=vs[:], scalar=-3.0,
                in1=Wm[:, 256:384], op0=ALU.add, op1=ALU.add)
            nc.gpsimd.dma_start(oseg[:, g * 256:(g + 1) * 256],
                                obuf_seg[:, g * 256:(g + 1) * 256])

        for g in range(G):
            if g == 0:
                emit_main(g)
                emit_seg(g)
            else:
                emit_seg(g)
                emit_main(g)

    nc.compile()
    return nc


_NC_CACHE = None


def _get_nc():
    global _NC_CACHE
    if _NC_CACHE is None:
        _NC_CACHE = build_nc()
    return _NC_CACHE


# ------------------------------------------------------------------ host side
def _norm8(x, scale):
    n = np.sqrt((x * x).sum(-1, keepdims=True))
    n = np.maximum(n, 1e-12)
    return (scale * x / n).astype(NP_F8)


def build_in_maps(support_features, target_features, support_labels,
                  logit_scale, fusion_logits):
    support_features = np.asarray(support_features, dtype=np.float32)
    target_features = np.asarray(target_features, dtype=np.float32)
    support_labels = np.asarray(support_labels, dtype=np.int32)

    # ---- prototypes (exact f32 scatter-mean, normalized, x16, fp8)
    proto = np.zeros((K, T, D), np.float32)
    cnt = np.zeros((K,), np.float32)
    np.add.at(proto, support_labels % K, support_features)
    np.add.at(cnt, support_labels % K, 1.0)
    proto /= cnt[:, None, None]
    p8 = _norm8(proto, PSC)                                   # [K, T, D]
    segp = np.stack([proto[:, s:e].sum(1) for s, e in WINDOWS], 1)
    sp8 = _norm8(segp, PSC)                                   # [K, NW, D]

    # pT: [p][(j,o,k,s)]
    pT_h = np.ascontiguousarray(
        p8.reshape(K, T, DJ, 2, 128).transpose(4, 2, 3, 0, 1)
    ).reshape(128, DJ * 2 * K * T)
    # sT: [p][(j,o,v,k)]
    sT_h = np.ascontiguousarray(
        sp8.reshape(K, NW, DJ, 2, 128).transpose(4, 2, 3, 1, 0)
    ).reshape(128, DJ * 2 * NW * K)

    # ---- queries: normalized x64, fp8, d-major
    q8 = _norm8(target_features, QSC)                         # [Q, T, D]
    segq = np.stack([target_features[:, s:e].sum(1) for s, e in WINDOWS], 1)
    sq8 = _norm8(segq, QSC)                                   # [Q, NW, D]

    in_maps = []
    for c in range(NCORES):
        x8 = q8[c * QPC:(c + 1) * QPC]
        tf_h = np.ascontiguousarray(
            x8.reshape(G, 128, T, DJ, 2, 128).transpose(0, 5, 3, 4, 2, 1)
        ).reshape(G, 128, DJ * 2 * T * 128)
        s8 = sq8[c * QPC:(c + 1) * QPC]
        tsg_h = np.ascontiguousarray(
            s8.reshape(G, 128, NW, DJ, 2, 128).transpose(0, 5, 3, 4, 2, 1)
        ).reshape(G, 128, DJ * 2 * NW * 128)
        in_maps.append({"tf": tf_h, "tsg": tsg_h, "pT": pT_h, "sT": sT_h})
    return in_maps


def kernel(support_features, target_features, support_labels, logit_scale,
           fusion_logits):
    logit_scale = np.asarray(logit_scale, dtype=np.float32)
    fusion_logits = np.asarray(fusion_logits, dtype=np.float32)
    in_maps = build_in_maps(support_features, target_features, support_labels,
                            logit_scale, fusion_logits)
    nc = _get_nc()
    res = run_bass_kernel_spmd(nc, in_maps, core_ids=list(range(NCORES)))

    glo = np.empty((Q, K), np.float32)
    s2q = np.empty((Q, K), np.float32)
    q2s = np.empty((Q, K), np.float32)
    for c in range(NCORES):
        og = np.asarray(res.results[c]["oglo"]).reshape(128, G, K)
        glo[c * QPC:(c + 1) * QPC] = og.transpose(1, 0, 2).reshape(QPC, K)
        os_ = np.asarray(res.results[c]["oseg"]).reshape(64, G, 2, 128)
        # [k, g, which, q] -> [g, q, k]
        s2q[c * QPC:(c + 1) * QPC] = os_[:, :, 0].transpose(1, 2, 0).reshape(
            QPC, K)
        q2s[c * QPC:(c + 1) * QPC] = os_[:, :, 1].transpose(1, 2, 0).reshape(
            QPC, K)

    e = np.exp(fusion_logits - fusion_logits.max())
    fw = (e / e.sum()) * np.exp(logit_scale)
    fused = fw[0] * glo + fw[1] * s2q + fw[2] * q2s
    return (fused.astype(np.float32), glo, s2q, q2s)


if __name__ == "__main__":
    rng = np.random.default_rng(0)
    ins = {
        "support_features": rng.standard_normal((S, T, D), dtype=np.float32),
        "target_features": rng.standard_normal((Q, T, D), dtype=np.float32),
        "support_labels": (np.arange(S) % K).astype(np.int32),
        "logit_scale": np.float32(0.0),
        "fusion_logits": np.zeros(3, np.float32),
    }
    outs = kernel(**ins)
    for o in outs:
        print(o.shape, o.dtype, float(o.mean()))


# revision 11
# speedup vs baseline: 1.2832x; 1.0144x over previous
"""Trainium2 Bass kernel for few-shot video retrieval (bidirectional chamfer
distance to class prototypes, global frame-level + segment-level, fused).

Contract: kernel(**inputs) takes the FULL unsharded inputs (numpy) and returns
the full outputs (tuple of 4 [4096, 64] float32 arrays), matching reference().

Sharding: data-parallel over the query axis across 8 NeuronCores; prototypes
(computed on host, like the norm factors) replicated. Gather + fusion on host.

Device-side algorithm per core (512 queries = 4 slices of 128):
  - host pre-normalizes every query frame (x64) and every prototype frame
    (x16) in f32, then casts to fp8 e4m3 -> all PSUM results are 1024*sim
    with a single constant drain scale; no per-(q,t) norm factors on device
  - main sims GEMM: queries stationary (d-major), protoT moving, fp8
    DoubleRow (256-deep contraction), output [q, (k, ts)] with ts innermost
  - chamfer: dir0 (max over ts) = two grouped tensor_reduce ops (contiguous
    innermost axis, 2x bf16); dir1 (max over tq) = pairwise bf16 max TTs that
    pipeline with the PSUM drains; sums via strided reduces
  - segments: 3 separate GEMM groups (one per support window v), stationary
    = seg prototypes [d, k] so outputs land k-major on partitions 0-63;
    chamfer trees split between DVE and GpSimd
  - fusion softmax/exp + final gather/transpose on host
"""

import sys

sys.path.insert(0, "/opt/trn_rl_repo")

import numpy as np
import ml_dtypes
from contextlib import ExitStack

import concourse.bass as bass
import concourse.bacc as bacc
import concourse.tile as tile
from concourse import mybir
from concourse.bass_utils import run_bass_kernel_spmd

# ---------------------------------------------------------------- problem dims
S, Q, T, D = 256, 4096, 8, 1024
K = 64                      # classes
NCORES = 8
QPC = Q // NCORES           # 512 queries per core
G = QPC // 128              # 4 query-slices of 128 per core
DJ = 4                      # 4 DoubleRow chunks (256-deep)
NW = 3                      # segment windows
WINDOWS = ((0, 4), (2, 6), (4, 8))
QSC = 64.0                  # query fp8 scale (host-normalized frames)
PSC = 16.0                  # prototype fp8 scale
ISC = 1.0 / (QSC * PSC)     # drain scale: PSUM value = 1024 * sim

F32 = mybir.dt.float32
BF16 = mybir.dt.bfloat16
F8 = mybir.dt.float8e4
AF = mybir.ActivationFunctionType
ALU = mybir.AluOpType
AX = mybir.AxisListType
DR = mybir.MatmulPerfMode.DoubleRow

NP_F8 = ml_dtypes.float8_e4m3


# ---------------------------------------------------------------- bass kernel
def build_nc():
    nc = bacc.Bacc("TRN2", target_bir_lowering=False, debug=False,
                   num_devices=NCORES)

    # d-major normalized queries: [g][p(d%128)][(t, j, o, q)] (t-major so
    # each tq matmul group depends only on its own 1KB/partition slice)
    tf = nc.dram_tensor("tf", [G, 128, DJ * 2 * T * 128], F8,
                        kind="ExternalInput")
    # d-major normalized query segments: [g][p][(j, o, w, q)]
    tsg = nc.dram_tensor("tsg", [G, 128, DJ * 2 * NW * 128], F8,
                         kind="ExternalInput")
    # d-major normalized frame prototypes: [p][(j, o, k, s)]
    pT = nc.dram_tensor("pT", [128, DJ * 2 * K * T], F8, kind="ExternalInput")
    # d-major normalized segment prototypes: [p][(j, o, v, k)]
    sT = nc.dram_tensor("sT", [128, DJ * 2 * NW * K], F8,
                        kind="ExternalInput")

    # outputs: -global_dist q-major; -(s2q|q2s) k-major
    oglo = nc.dram_tensor("oglo", [128, G * K], F32, kind="ExternalOutput")
    oseg = nc.dram_tensor("oseg", [64, G * 2 * 128], F32,
                          kind="ExternalOutput")

    NWARM = 16

    with tile.TileContext(nc) as tc, ExitStack() as ctx:
        const = ctx.enter_context(tc.tile_pool(name="const", bufs=1))
        persist = ctx.enter_context(tc.tile_pool(name="persist", bufs=1))
        simpool = ctx.enter_context(tc.tile_pool(name="simpool", bufs=2))
        segpool = ctx.enter_context(tc.tile_pool(name="segpool", bufs=2))
        work = ctx.enter_context(tc.tile_pool(name="work", bufs=2))

        # ---------------- PE warmup burst (HAM ramp while inputs stream in);
        # memset is the first gpsimd instruction so it lands ~instantly
        wz = const.tile([128, 256], F8)
        nc.gpsimd.memset(wz[:], 0)
        with tc.tile_pool(name="psW", bufs=1, space="PSUM") as psW:
            wps = psW.tile([128, 256], F32)
            for _ in range(NWARM):
                nc.tensor.matmul(wps[:], wz[:, 0:128], wz[:], start=True,
                                 stop=True)

        # ---------------- input DMAs (issue order = priority order)
        pT_t = const.tile([128, DJ * 2 * K * T], F8)
        for h in range(2):
            cols = slice(h * 2048, (h + 1) * 2048)
            nc.gpsimd.dma_start(pT_t[:, cols], pT[:, cols])
        tf_t = []
        tsg_t = []
        for g in range(G):
            tf_t.append(const.tile([128, DJ * 2 * T * 128], F8,
                                   name=f"tf{g}"))
            tsg_t.append(const.tile([128, DJ * 2 * NW * 128], F8,
                                    name=f"tsg{g}"))
        for h in range(4):
            cols = slice(h * 2048, (h + 1) * 2048)
            nc.gpsimd.dma_start(tf_t[0][:, cols], tf[0][:, cols])
        sT_t = const.tile([128, DJ * 2 * NW * K], F8)
        nc.gpsimd.dma_start(sT_t[:], sT[:])
        nc.gpsimd.dma_start(tsg_t[0][:], tsg[0])
        for h in range(2):
            cols = slice(h * 4096, (h + 1) * 4096)
            nc.gpsimd.dma_start(tf_t[1][:, cols], tf[1][:, cols])
        nc.gpsimd.dma_start(tsg_t[1][:], tsg[1])
        nc.gpsimd.dma_start(tsg_t[2][:], tsg[2])
        for h in range(2):
            cols = slice(h * 4096, (h + 1) * 4096)
            nc.gpsimd.dma_start(tf_t[2][:, cols], tf[2][:, cols])
        nc.gpsimd.dma_start(tsg_t[3][:], tsg[3])
        for h in range(2):
            cols = slice(h * 4096, (h + 1) * 4096)
            nc.gpsimd.dma_start(tf_t[3][:, cols], tf[3][:, cols])

        pT_v = pT_t[:].rearrange("p (j o k s) -> p j o k s", j=DJ, o=2, k=K)
        sT_v = sT_t[:].rearrange("p (j o v k) -> p j o v k", j=DJ, o=2, v=NW)

        obuf_glo = persist.tile([128, G * K], F32)
        obuf_seg = persist.tile([64, G * 2 * 128], F32)

        psM = ctx.enter_context(tc.tile_pool(name="psM", bufs=3,
                                             space="PSUM"))
        psS = ctx.enter_context(tc.tile_pool(name="psS", bufs=1,
                                             space="PSUM"))

        def emit_main(g):
            tf_v = tf_t[g][:].rearrange("p (t j o q) -> p t j o q", t=T,
                                        j=DJ, o=2)
            simcp = simpool.tile([128, T * K * T], BF16, tag="simcp")
            pmax = simpool.tile([128, 4 * K * T], BF16, tag="pmax")
            Lh = simpool.tile([128, 2 * 4 * K * 4], BF16, tag="Lh")
            for tq in range(T):
                mp = psM.tile([128, K * T], F32, tag="mp")
                for j in range(DJ):
                    nc.tensor.matmul(mp[:], tf_v[:, tq, j, :, :],
                                     pT_v[:, j], start=(j == 0),
                                     stop=(j == DJ - 1), perf_mode=DR)
                nc.scalar.activation(simcp[:, tq * 512:(tq + 1) * 512],
                                     mp[:], AF.Copy, scale=ISC)
                if tq % 2 == 1:
                    i = tq // 2
                    nc.vector.tensor_tensor(
                        pmax[:, i * 512:(i + 1) * 512],
                        simcp[:, (tq - 1) * 512:tq * 512],
                        simcp[:, tq * 512:(tq + 1) * 512], ALU.max)
                if tq == 3 or tq == 7:
                    # dir1 half-merge + dir0 s-halving L1 (hide under MMs)
                    h = tq // 4
                    Th = work.tile([128, 512], BF16, tag=f"T{h}")
                    nc.vector.tensor_tensor(
                        Th[:], pmax[:, h * 1024:h * 1024 + 512],
                        pmax[:, h * 1024 + 512:h * 1024 + 1024], ALU.max)
                    if h == 0:
                        T1 = Th
                    else:
                        T2 = Th
                    sh = simcp[:, h * 2048:(h + 1) * 2048].rearrange(
                        "p (tk s) -> p tk s", s=T)
                    nc.vector.tensor_tensor(
                        Lh[:, h * 1024:(h + 1) * 1024].rearrange(
                            "p (tk s) -> p tk s", s=4),
                        sh[:, :, 0:4], sh[:, :, 4:8], ALU.max)
            Rm = work.tile([128, 512], BF16, tag="Rm")
            nc.vector.tensor_tensor(Rm[:], T1[:], T2[:], ALU.max)
            msum = work.tile([128, K], F32, tag="msum")
            nc.vector.tensor_reduce(msum[:],
                                    Rm[:].rearrange("p (k s) -> p k s", k=K),
                                    axis=AX.X, op=ALU.add)
            # dir0 L2/L3: (h,t,k,s4) -> (h,t,k)
            L2 = work.tile([128, 1024], BF16, tag="L2")
            lhv = Lh[:].rearrange("p (tk s) -> p tk s", s=4)
            nc.vector.tensor_tensor(
                L2[:].rearrange("p (tk s) -> p tk s", s=2),
                lhv[:, :, 0:2], lhv[:, :, 2:4], ALU.max)
            Am = work.tile([128, 512], BF16, tag="Am")
            l2v = L2[:].rearrange("p (tk s) -> p tk s", s=2)
            nc.vector.tensor_tensor(Am[:].rearrange("p (tk s) -> p tk s",
                                                    s=1),
                                    l2v[:, :, 0:1], l2v[:, :, 1:2], ALU.max)
            # asum tree over t: Am layout (h2, t4, k64)
            h1 = work.tile([128, 256], BF16, tag="h1")
            nc.vector.tensor_tensor(h1[:], Am[:, 0:256], Am[:, 256:512],
                                    ALU.add)
            h2 = work.tile([128, 128], BF16, tag="h2")
            nc.vector.tensor_tensor(h2[:], h1[:, 0:128], h1[:, 128:256],
                                    ALU.add)
            asum = work.tile([128, K], F32, tag="asum")
            nc.vector.tensor_tensor(asum[:], h2[:, 0:64], h2[:, 64:128],
                                    ALU.add)
            nc.vector.scalar_tensor_tensor(
                obuf_glo[:, g * K:(g + 1) * K], in0=asum[:], scalar=-16.0,
                in1=msum[:], op0=ALU.add, op1=ALU.add)
            nc.gpsimd.dma_start(oglo[:, g * K:(g + 1) * K],
                                obuf_glo[:, g * K:(g + 1) * K])

        def emit_seg(g):
            tsg_v = tsg_t[g][:].rearrange("p (j o w q) -> p j o w q", j=DJ,
                                          o=2, w=NW)
            segs = segpool.tile([64, NW * NW * 128], BF16, tag="segs")
            for v in range(NW):
                sp = psS.tile([64, NW * 128], F32, tag=f"sv{v}")
                for j in range(DJ):
                    nc.tensor.matmul(sp[:], sT_v[:, j, :, v, :],
                                     tsg_v[:, j], start=(j == 0),
                                     stop=(j == DJ - 1), perf_mode=DR)
                nc.scalar.activation(segs[:, v * 384:(v + 1) * 384], sp[:],
                                     AF.Copy, scale=ISC)
            # q2s = sum_w max_v  (contiguous 384-wide maxes over v-slices)
            m01 = work.tile([64, NW * 128], BF16, tag="m01")
            nc.vector.tensor_tensor(m01[:], segs[:, 0:384], segs[:, 384:768],
                                    ALU.max)
            m012 = work.tile([64, NW * 128], BF16, tag="m012")
            nc.vector.tensor_tensor(m012[:], m01[:], segs[:, 768:1152],
                                    ALU.max)
            ws = work.tile([64, 128], BF16, tag="ws")
            nc.vector.tensor_tensor(ws[:], m012[:, 0:128], m012[:, 128:256],
                                    ALU.add)
            nc.vector.scalar_tensor_tensor(
                obuf_seg[:, g * 256 + 128:g * 256 + 256], in0=ws[:],
                scalar=-3.0, in1=m012[:, 256:384], op0=ALU.add, op1=ALU.add)
            # s2q = sum_v max_w  (strided views over w, v in the free dim)
            vv = segs[:].rearrange("p (v w q) -> p v w q", v=NW, w=NW)
            W1 = work.tile([64, NW * 128], BF16, tag="W1")
            w1v = W1[:].rearrange("p (v q) -> p v q", v=NW)
            nc.vector.tensor_tensor(w1v, vv[:, :, 0, :], vv[:, :, 1, :],
                                    ALU.max)
            Wm = work.tile([64, NW * 128], BF16, tag="Wm")
            wmv = Wm[:].rearrange("p (v q) -> p v q", v=NW)
            nc.vector.tensor_tensor(wmv, w1v, vv[:, :, 2, :], ALU.max)
            vs = work.tile([64, 128], BF16, tag="vs")
            nc.vector.tensor_tensor(vs[:], Wm[:, 0:128], Wm[:, 128:256],
                                    ALU.add)
            nc.vector.scalar_tensor_tensor(
                obuf_seg[:, g * 256:g * 256 + 128], in0=vs[:], scalar=-3.0,
                in1=Wm[:, 256:384], op0=ALU.add, op1=ALU.add)
            nc.gpsimd.dma_start(oseg[:, g * 256:(g + 1) * 256],
                                obuf_seg[:, g * 256:(g + 1) * 256])

        for g in range(G):
            emit_main(g)
            emit_seg(g)

    nc.compile()
    return nc


_NC_CACHE = None


def _get_nc():
    global _NC_CACHE
    if _NC_CACHE is None:
        _NC_CACHE = build_nc()
    return _NC_CACHE


# ------------------------------------------------------------------ host side
def _norm8(x, scale):
    n = np.sqrt((x * x).sum(-1, keepdims=True))
    n = np.maximum(n, 1e-12)
    return (scale * x / n).astype(NP_F8)


def build_in_maps(support_features, target_features, support_labels,
                  logit_scale, fusion_logits):
    support_features = np.asarray(support_features, dtype=np.float32)
    target_features = np.asarray(target_features, dtype=np.float32)
    support_labels = np.asarray(support_labels, dtype=np.int32)

    # ---- prototypes (exact f32 scatter-mean, normalized, x16, fp8)
    proto = np.zeros((K, T, D), np.float32)
    cnt = np.zeros((K,), np.float32)
    np.add.at(proto, support_labels % K, support_features)
    np.add.at(cnt, support_labels % K, 1.0)
    proto /= cnt[:, None, None]
    p8 = _norm8(proto, PSC)                                   # [K, T, D]
    segp = np.stack([proto[:, s:e].sum(1) for s, e in WINDOWS], 1)
    sp8 = _norm8(segp, PSC)                                   # [K, NW, D]

    # pT: [p][(j,o,k,s)]
    pT_h = np.ascontiguousarray(
        p8.reshape(K, T, DJ, 2, 128).transpose(4, 2, 3, 0, 1)
    ).reshape(128, DJ * 2 * K * T)
    # sT: [p][(j,o,v,k)]
    sT_h = np.ascontiguousarray(
        sp8.reshape(K, NW, DJ, 2, 128).transpose(4, 2, 3, 1, 0)
    ).reshape(128, DJ * 2 * NW * K)

    # ---- queries: normalized x64, fp8, d-major
    q8 = _norm8(target_features, QSC)                         # [Q, T, D]
    segq = np.stack([target_features[:, s:e].sum(1) for s, e in WINDOWS], 1)
    sq8 = _norm8(segq, QSC)                                   # [Q, NW, D]

    in_maps = []
    for c in range(NCORES):
        x8 = q8[c * QPC:(c + 1) * QPC]
        tf_h = np.ascontiguousarray(
            x8.reshape(G, 128, T, DJ, 2, 128).transpose(0, 5, 2, 3, 4, 1)
        ).reshape(G, 128, DJ * 2 * T * 128)
        s8 = sq8[c * QPC:(c + 1) * QPC]
        tsg_h = np.ascontiguousarray(
            s8.reshape(G, 128, NW, DJ, 2, 128).transpose(0, 5, 3, 4, 2, 1)
        ).reshape(G, 128, DJ * 2 * NW * 128)
        in_maps.append({"tf": tf_h, "tsg": tsg_h, "pT": pT_h, "sT": sT_h})
    return in_maps


def kernel(support_features, target_features, support_labels, logit_scale,
           fusion_logits):
    logit_scale = np.asarray(logit_scale, dtype=np.float32)
    fusion_logits = np.asarray(fusion_logits, dtype=np.float32)
    in_maps = build_in_maps(support_features, target_features, support_labels,
                            logit_scale, fusion_logits)
    nc = _get_nc()
    res = run_bass_kernel_spmd(nc, in_maps, core_ids=list(range(NCORES)))

    glo = np.empty((Q, K), np.float32)
    s2q = np.empty((Q, K), np.float32)
    q2s = np.empty((Q, K), np.float32)
    for c in range(NCORES):
        og = np.asarray(res.results[c]["oglo"]).reshape(128, G, K)
        glo[c * QPC:(c + 1) * QPC] = og.transpose(1, 0, 2).reshape(QPC, K)
        os_ = np.asarray(res.results[c]["oseg"]).reshape(64, G, 2, 128)
        # [k, g, which, q] -> [g, q, k]
        s2q[c * QPC:(c + 1) * QPC] = os_[:, :, 0].transpose(1, 2, 0).reshape(
            QPC, K)
        q2s[c * QPC:(c + 1) * QPC] = os_[:, :, 1].transpose(1, 2, 0).reshape(
            QPC, K)

    e = np.exp(fusion_logits - fusion_logits.max())
    fw = (e / e.sum()) * np.exp(logit_scale)
    fused = fw[0] * glo + fw[1] * s2q + fw[2] * q2s
    return (fused.astype(np.float32), glo, s2q, q2s)


if __name__ == "__main__":
    rng = np.random.default_rng(0)
    ins = {
        "support_features": rng.standard_normal((S, T, D), dtype=np.float32),
        "target_features": rng.standard_normal((Q, T, D), dtype=np.float32),
        "support_labels": (np.arange(S) % K).astype(np.int32),
        "logit_scale": np.float32(0.0),
        "fusion_logits": np.zeros(3, np.float32),
    }
    outs = kernel(**ins)
    for o in outs:
        print(o.shape, o.dtype, float(o.mean()))
